# revision 15
# baseline (speedup 1.0000x reference)
"""Low-rank bilinear attention kernel for Trainium2 (Bass/Tile), 8 NeuronCores.

Math: alpha[b,l,p] = sum_c wt_c * (sum_a tanh(p1[b,p,a]*p2[b,l,a]) * Wh[c,a] + bh_c) + bt
    = sum_a v_a * tanh(p1[b,p,a]*p2[b,l,a]) + const
  with v = wt @ Wh (weight fold), const = wt @ bh + bt (added on host).
  p1 = x1 @ W1.T, p2 = x2 @ W2.T.

Sharding: data-parallel over B (8 batches -> 8 cores). Weights replicated.

Device layout per core: A (1024) split into 8 blocks of 128 on partitions;
loop j (A-block) outer, g (label group of 20) inner. Free dim of the main
tiles is (p,l)-major: column index p*20+l. This keeps every DVE operand's
innermost AP stride at +-1, which is required for the DVE 2x mode:
  p1rep[j][:, p*20+l] = p1T[j][:, p]   (one DVE broadcast copy per j,
                                        read straight from the proj PSUM)
  m = p1rep * p2T[j, g-slice]  -- in1 AP [[0,196],[1,20]], bf16, 2x mode
  h = tanh(m) on ACT (the bottleneck engine: 32 x 3.56us)
  contraction over A: bf16 matmuls with zero-padded v strips (lhsT [128,8],
  only column c nonzero) accumulating into 4 live [8,490] PSUM tiles (one
  per group, j=0 starts, j=7 stops); one DVE copy + DMA per group.
Host post-pass adds const and un-permutes the (p,l)-major output.
"""

import os
import sys

import numpy as np

if "/opt/trn_rl_repo" not in sys.path:
    sys.path.insert(0, "/opt/trn_rl_repo")

import concourse.bass as bass
from concourse import bacc
import concourse.mybir as mybir
from concourse.bass import AP
from concourse.bass_utils import run_bass_kernel_spmd
from concourse.tile import TileContext

B, P, L = 8, 196, 80
D1, D2, A = 2048, 300, 1024
NBLK = A // 128          # 8 A-blocks
ND1 = D1 // 128          # 16 d-chunks for W1
D2P = 384                # D2 padded to 3*128
ND2 = D2P // 128         # 3
G = 20                   # labels per group
NG = L // G              # 4 groups
GW = G * P               # 3920 free width of one group tile
NCH = 8                  # contraction chunks per group
CW = GW // NCH           # 490 columns per chunk (fits one PSUM bank)
VS = 15                  # width of one zero-padded v strip

F32 = mybir.dt.float32
BF16 = mybir.dt.bfloat16
TANH = mybir.ActivationFunctionType.Tanh

_LAST_PERF = {}


def _build():
    nc = bacc.Bacc(None, target_bir_lowering=False)

    x1_d = nc.declare_dram_parameter("x1t", [128, ND1 * P], BF16,
                                     isOutput=False)
    w1_d = nc.declare_dram_parameter("w1r", [A, D1], BF16, isOutput=False)
    x2_d = nc.declare_dram_parameter("x2t", [128, ND2 * L], F32,
                                     isOutput=False)
    w2_d = nc.declare_dram_parameter("w2r", [A, D2P], F32, isOutput=False)
    vz_d = nc.declare_dram_parameter("vzd", [128, NBLK * VS], BF16,
                                     isOutput=False)
    out_d = nc.declare_dram_parameter("alpha", [NG * GW], F32, isOutput=True)

    with TileContext(nc) as tc:
        with (
            tc.tile_pool(name="const", bufs=1) as cpool,
            tc.tile_pool(name="w1", bufs=3) as w1p,
            tc.tile_pool(name="w2", bufs=2) as w2p,
            tc.tile_pool(name="p1rep", bufs=2) as rp,
            tc.tile_pool(name="m", bufs=3) as mp,
            tc.tile_pool(name="tanh", bufs=3) as hp,
            tc.tile_pool(name="alphas", bufs=2) as alp,
        ):
            # Warm the ACT tanh table first thing.
            warm = cpool.tile([1, 2], F32)
            nc.vector.memset(warm[:, :], 0.0)
            nc.scalar.activation(warm[:, :], warm[:, :], TANH)

            # Inputs
            x1T = cpool.tile([128, ND1 * P], BF16)
            nc.sync.dma_start(out=x1T[:, :], in_=x1_d[:, :])
            x2T = cpool.tile([128, ND2 * L], F32)
            nc.sync.dma_start(out=x2T[:, :], in_=x2_d[:, :])
            vzr = cpool.tile([128, NBLK * VS], BF16)
            nc.sync.dma_start(out=vzr[:, :], in_=vz_d[:, :])

            p2T = cpool.tile([128, NBLK * L], BF16)

            with (
                tc.tile_pool(name="ps_p2", bufs=1, space="PSUM") as ps2,
                tc.tile_pool(name="ps_p1", bufs=2, space="PSUM") as ps1,
                tc.tile_pool(name="ps_al", bufs=1, space="PSUM") as psa,
            ):
                def p2blk(j):
                    w2_sb = w2p.tile([128, D2P], F32, tag="w2",
                                     name=f"w2sb_{j}")
                    nc.sync.dma_start(out=w2_sb[:, :],
                                      in_=w2_d[j * 128:(j + 1) * 128, :])
                    pm = ps2.tile([128, L], F32, tag="p2ps",
                                  name=f"p2ps_{j}")
                    for kk in range(ND2):
                        nc.tensor.matmul(pm[:, :],
                                         lhsT=w2_sb[:, kk * 128:(kk + 1) * 128],
                                         rhs=x2T[:, kk * L:(kk + 1) * L],
                                         start=(kk == 0), stop=(kk == ND2 - 1))
                    nc.vector.tensor_copy(p2T[:, j * L:(j + 1) * L], pm[:, :])

                def proj(j):
                    w1_sb = w1p.tile([128, D1], BF16, tag="w1",
                                     name=f"w1sb_{j}")
                    nc.sync.dma_start(out=w1_sb[:, :],
                                      in_=w1_d[j * 128:(j + 1) * 128, :])
                    pm1 = ps1.tile([128, P], F32, tag="p1ps",
                                   name=f"p1ps_{j}")
                    for k in range(ND1):
                        nc.tensor.matmul(pm1[:, :],
                                         lhsT=w1_sb[:, k * 128:(k + 1) * 128],
                                         rhs=x1T[:, k * P:(k + 1) * P],
                                         start=(k == 0), stop=(k == ND1 - 1))
                    return pm1

                proj_ps = {0: proj(0)}
                p2blk(0)
                proj_ps[1] = proj(1)

                al_ps = [psa.tile([NCH, CW], F32, tag=f"al{g}",
                                  name=f"alps_{g}")
                         for g in range(NG)]

                for j in range(NBLK):
                    # p1rep[:, p*G+l] = proj_psum[:, p], bf16 out
                    pm1 = proj_ps.pop(j)
                    p1rep = rp.tile([128, GW], BF16, tag="p1rep")
                    a = pm1[:, :]
                    rin = AP(a.tensor, a.offset, [a.ap[0], [1, P], [0, G]])
                    ao = p1rep[:, :]
                    rout = AP(ao.tensor, ao.offset, [ao.ap[0], [G, P], [1, G]])
                    nc.vector.tensor_copy(rout, rin)
                    if j + 2 < NBLK:
                        proj_ps[j + 2] = proj(j + 2)

                    for g in range(NG):
                        m = mp.tile([128, GW], BF16, tag="m")
                        x = p2T[:, j * L + g * G:j * L + g * G + G]
                        in1 = AP(x.tensor, x.offset,
                                 [x.ap[0], [0, P], [1, G]])
                        nc.vector.tensor_tensor(m[:, :], p1rep[:, :], in1,
                                                mybir.AluOpType.mult)
                        h = hp.tile([128, GW], BF16, tag="h")
                        nc.scalar.activation(h[:, :], m[:, :], TANH)
                        for c in range(NCH):
                            nc.tensor.matmul(
                                al_ps[g][:, :],
                                lhsT=vzr[:, j * VS + 7 - c:j * VS + VS - c],
                                rhs=h[:, c * CW:(c + 1) * CW],
                                start=(j == 0 and c == 0),
                                stop=(j == NBLK - 1 and c == NCH - 1))
                        if j == NBLK - 1:
                            alpha_sb = alp.tile([NCH, CW], F32, tag="alpha",
                                                name=f"alpha_{g}")
                            nc.vector.tensor_copy(alpha_sb[:, :],
                                                  al_ps[g][:, :])
                            nc.sync.dma_start(out=out_d[g * GW:(g + 1) * GW],
                                              in_=alpha_sb[:, :])
                    if j + 1 < NBLK:
                        p2blk(j + 1)
    nc.finalize()
    return nc


def _install_axon_trace_hook() -> bool:
    """Install the NTFF profiling hook for axon runs (test-time only)."""
    try:
        import contextlib
        import ctypes
        import types

        so_path = "/opt/axon/libaxon_pjrt.so"
        if not os.path.exists(so_path):
            return False
        lib = ctypes.CDLL(so_path)
        if not hasattr(lib, "axon_start_nrt_profile"):
            return False
        lib.axon_start_nrt_profile.argtypes = [
            ctypes.POINTER(ctypes.c_int64), ctypes.c_size_t]
        lib.axon_start_nrt_profile.restype = ctypes.c_int64
        lib.axon_stop_nrt_profile.argtypes = [ctypes.c_char_p]
        lib.axon_stop_nrt_profile.restype = ctypes.c_int64

        @contextlib.contextmanager
        def _hook(output_dir, device_ids):
            import jax
            jax.devices()
            if device_ids:
                ids = (ctypes.c_int64 * len(device_ids))(*device_ids)
                rc = lib.axon_start_nrt_profile(ids, len(device_ids))
            else:
                rc = lib.axon_start_nrt_profile(None, 0)
            if rc != 0:
                raise RuntimeError(f"axon_start_nrt_profile rc={rc}")
            try:
                yield
            finally:
                n = lib.axon_stop_nrt_profile(str(output_dir).encode())
                print(f"profile: {n} file(s) written to {output_dir}",
                      file=sys.stderr)

        mod = types.ModuleType("antenv.axon_hooks")
        mod.get_axon_ntff_profile_hook = lambda: _hook
        mod.set_axon_ntff_profile_hook = lambda h: None
        sys.modules["antenv.axon_hooks"] = mod

        import concourse.bass_utils as bu
        bu.upload_artifacts = lambda tmpdir: f"local://{tmpdir}"
        return True
    except Exception as e:  # pragma: no cover
        print(f"trace hook install failed: {e}", file=sys.stderr)
        return False


def kernel(x1, x2, W1, W2, Wh, bh, wt, bt):
    import ml_dtypes

    x1 = np.ascontiguousarray(np.asarray(x1, dtype=np.float32))
    x2 = np.ascontiguousarray(np.asarray(x2, dtype=np.float32))
    W1 = np.asarray(W1, dtype=np.float32)
    W2 = np.asarray(W2, dtype=np.float32)
    Wh = np.asarray(Wh, dtype=np.float32)
    bh = np.asarray(bh, dtype=np.float32)
    wt = np.asarray(wt, dtype=np.float32)
    bt = np.float32(np.asarray(bt))

    # Weight folding (host): rank-1 output head collapses into v.
    v = wt @ Wh                                   # [A]
    const_val = np.float32(wt @ bh + np.float32(bt))

    # w1 blocks: w1r[j*128+d, k*128+a] = W1[j*128+a, k*128+d]  (bf16)
    w1r = np.ascontiguousarray(
        W1.reshape(NBLK, 128, ND1, 128).transpose(0, 3, 2, 1).reshape(A, D1)
        .astype(ml_dtypes.bfloat16))
    # w2 blocks: w2r[j*128+d, kk*128+a] = W2[j*128+a, kk*128+d]  (fp32, padded)
    w2tp = np.zeros((D2P, A), dtype=np.float32)
    w2tp[:D2] = W2.T
    w2r = np.ascontiguousarray(
        w2tp.reshape(ND2, 128, NBLK, 128).transpose(2, 1, 0, 3).reshape(A, D2P))
    # zero-padded v strips: vzd[:, j*VS+7] = v block j, else 0.
    vzd = np.zeros((128, NBLK * VS), dtype=np.float32)
    for j in range(NBLK):
        vzd[:, j * VS + 7] = v[j * 128:(j + 1) * 128]
    vzd = np.ascontiguousarray(vzd.astype(ml_dtypes.bfloat16))

    nc = _build()

    # Host pre-transposes (layout only).
    x2tp = np.zeros((D2P, L), dtype=np.float32)
    in_maps = []
    for b in range(B):
        x1t = np.ascontiguousarray(
            x1[b].T.reshape(ND1, 128, P).transpose(1, 0, 2).reshape(128, -1)
            .astype(ml_dtypes.bfloat16))
        x2tp[:D2] = x2[b].T
        x2t = np.ascontiguousarray(
            x2tp.reshape(ND2, 128, L).transpose(1, 0, 2).reshape(128, -1))
        in_maps.append({
            "x1t": x1t,
            "x2t": x2t,
            "w1r": w1r,
            "w2r": w2r,
            "vzd": vzd,
        })

    trace = os.environ.get("KERNEL_TRACE", "0") == "1"
    if trace:
        trace = _install_axon_trace_hook()
    res = run_bass_kernel_spmd(nc, in_maps, list(range(B)), trace=trace,
                               tmpdir=os.environ.get("KERNEL_TMPDIR") or None)
    _LAST_PERF.clear()
    _LAST_PERF["exec_time_ns"] = res.exec_time_ns
    _LAST_PERF["profile_json"] = res.profile_json

    # Un-permute: group g is [196, 20] (p-major) -> alpha rows g*20..g*20+19.
    out = np.empty((B, L, P), dtype=np.float32)
    for b in range(B):
        flat = res.results[b]["alpha"]
        for g in range(NG):
            out[b, g * G:(g + 1) * G, :] = \
                flat[g * GW:(g + 1) * GW].reshape(P, G).T
    out += const_val
    return out
